# revision 33
# baseline (speedup 1.0000x reference)
"""ErrorAwareEdgeLoss Trainium2 kernel (v5: local_scatter S-matrix).

Math: loss = mean_b [ (sum_e w_be * P[b,i_e,:] @ D @ P[b,j_e,:]) / max(sum_e w_be, 1e-8) ]

Reformulation:
    G_b = (P_b @ D) @ P_b^T               (two 256^3 matmuls on the PE, bf16)
    numerator_b = sum_e w_e * G_b[i_e, j_e] = <S_b, G_b>
    where S_b[i, j] = sum of w_e over edges with (i_e, j_e) = (i, j).

S_b is built ON-CHIP by gpsimd `local_scatter` (per-partition scatter through
Q7 local RAM, ~2us per 128x224-index call) in G's natural layout:
    S[p = i & 127, (i >> 7) * 256 + j] = w
local_scatter cannot accumulate duplicates, so edges are rank-split by
occurrence count of their exact (i, j): rank-0 edges go to table S1,
rank-1 to table S2 (separate scatter passes), and the rare rank>=2 edges
(~21 per batch) are handled on the gather side: one Pool-native
`indirect_copy` fetches G values for them from the bf16 G table and a
masked multiply adds their contribution. The scatters depend only on
host-prepared inputs, so they overlap the matmul pipeline entirely.

numerator = sum over cells of (S1+S2)*G  (DVE, 512 cells/partition/batch)
          + sum over leftover slots of w*gathered.

Sharding: data-parallel over batch: 8 NeuronCores x 8 batches. Each core
emits a partial sum of per-sample losses; the host adds the 8 partials and
divides by B (the all-reduce of the sharding hint).
"""

from contextlib import ExitStack

import numpy as np

import concourse.bacc as bacc
import concourse.bass as bass
import concourse.mybir as mybir
import concourse.tile as tile
from concourse.bass_utils import run_bass_kernel_spmd

B, N, E = 64, 256, 8192
NCORES = 8
BPC = B // NCORES  # batches per core
Q = E // 128  # ew free dim per partition (64)

NI1 = 112  # pass-1 scatter slots per partition per batch
NI2 = 24   # pass-2 slots per partition per batch
NL = 128   # leftover gather slots per 16-partition group (all batches)

f32 = mybir.dt.float32
bf16 = mybir.dt.bfloat16
i16 = mybir.dt.int16
u16 = mybir.dt.uint16


def _build_bass(ni1=None, ni2=None, nl=None):
    ni1, ni2, nl = ni1 or NI1, ni2 or NI2, nl or NL
    nc = bacc.Bacc("TRN2", target_bir_lowering=False, debug=False)

    pt_in = nc.dram_tensor("pt", [BPC, 128, 2, N], f32, kind="ExternalInput")
    d_in = nc.dram_tensor("derr", [128, 2, N], f32, kind="ExternalInput")
    ew_in = nc.dram_tensor("ew", [128, BPC, Q], f32, kind="ExternalInput")
    ix1_in = nc.dram_tensor("ix1", [128, BPC, ni1], i16, kind="ExternalInput")
    dt1_in = nc.dram_tensor("dt1", [128, BPC, ni1], f32, kind="ExternalInput")
    ix2_in = nc.dram_tensor("ix2", [128, BPC, ni2], i16, kind="ExternalInput")
    dt2_in = nc.dram_tensor("dt2", [128, BPC, ni2], f32, kind="ExternalInput")
    ixl_in = nc.dram_tensor("ixl", [128, nl // 16], i16, kind="ExternalInput")
    wl_in = nc.dram_tensor("wl", [128, BPC, nl], f32, kind="ExternalInput")
    out = nc.dram_tensor("out", [1, 1], f32, kind="ExternalOutput")

    with tile.TileContext(nc) as tc, ExitStack() as ctx:
        const_pool = ctx.enter_context(tc.tile_pool(name="const", bufs=1))
        pt_pool = ctx.enter_context(tc.tile_pool(name="pt", bufs=3))
        ptb_pool = ctx.enter_context(tc.tile_pool(name="ptb", bufs=3))
        qt_pool = ctx.enter_context(tc.tile_pool(name="qt", bufs=2))
        prod_pool = ctx.enter_context(tc.tile_pool(name="prod", bufs=2))
        psum_pool = ctx.enter_context(tc.tile_pool(name="ps", bufs=2, space="PSUM"))

        # ---- constants / whole-run tensors
        d_sb = const_pool.tile([128, 2, N], f32)
        nc.sync.dma_start(d_sb[:], d_in[:])
        db = const_pool.tile([128, 2, N], bf16)
        nc.vector.tensor_copy(db[:], d_sb[:])
        ones_sb = const_pool.tile([128, 1], f32)
        nc.vector.memset(ones_sb[:], 1.0)
        # cols [0,BPC): scatter numerators, [BPC,2B): leftover numerators,
        # [2B,3B): denominators
        red_sb = const_pool.tile([128, 3 * BPC], f32)

        ix1 = const_pool.tile([128, BPC, ni1], i16)
        dt1 = const_pool.tile([128, BPC, ni1], f32)
        ix2 = const_pool.tile([128, BPC, ni2], i16)
        dt2 = const_pool.tile([128, BPC, ni2], f32)
        ixl = const_pool.tile([128, nl // 16], i16)
        wl = const_pool.tile([128, BPC, nl], f32)
        ew_sb = const_pool.tile([128, BPC, Q], f32)
        pt_all = const_pool.tile([128, BPC, 2, N], f32)
        nc.sync.dma_start(pt_all[:, 0], pt_in[0])
        nc.sync.dma_start(ix1[:], ix1_in[:])
        nc.sync.dma_start(dt1[:], dt1_in[:])
        nc.sync.dma_start(ix2[:], ix2_in[:])
        nc.sync.dma_start(dt2[:], dt2_in[:])
        for b in range(1, BPC):
            nc.sync.dma_start(pt_all[:, b], pt_in[b])
        nc.sync.dma_start(ew_sb[:], ew_in[:])
        nc.sync.dma_start(ixl[:], ixl_in[:])
        nc.sync.dma_start(wl[:], wl_in[:])

        ptb_first = ptb_pool.tile([128, 2, N], bf16)
        nc.vector.tensor_copy(ptb_first[:], pt_all[:, 0])
        wlb = const_pool.tile([128, BPC, nl], bf16)
        db1 = const_pool.tile([128, BPC, ni1], bf16)
        nc.vector.tensor_copy(db1[:], dt1[:])
        db2 = const_pool.tile([128, BPC, ni2], bf16)
        nc.vector.tensor_copy(db2[:], dt2[:])

        # ---- scatter passes (input-only; overlap the matmul pipeline)
        s1 = const_pool.tile([128, BPC, 512], bf16)
        s2 = const_pool.tile([128, BPC, 512], bf16)
        for c in range(BPC // 2):
            nc.gpsimd.local_scatter(
                out_ap=s1[:, 2 * c : 2 * c + 2].rearrange("p b m -> p (b m)"),
                data_ap=db1[:, 2 * c : 2 * c + 2].rearrange("p b m -> p (b m)"),
                idxs_ap=ix1[:, 2 * c : 2 * c + 2].rearrange("p b m -> p (b m)"),
                channels=128,
                num_elems=1024,
                num_idxs=2 * ni1,
            )
            nc.gpsimd.local_scatter(
                out_ap=s2[:, 2 * c : 2 * c + 2].rearrange("p b m -> p (b m)"),
                data_ap=db2[:, 2 * c : 2 * c + 2].rearrange("p b m -> p (b m)"),
                idxs_ap=ix2[:, 2 * c : 2 * c + 2].rearrange("p b m -> p (b m)"),
                channels=128,
                num_elems=1024,
                num_idxs=2 * ni2,
            )
            # S += S2 for this 2-batch slice (in place)
            nc.vector.tensor_tensor(
                out=s1[:, 2 * c : 2 * c + 2],
                in0=s1[:, 2 * c : 2 * c + 2],
                in1=s2[:, 2 * c : 2 * c + 2],
                op=mybir.AluOpType.add,
            )

        nc.gpsimd.drain()

        tab = const_pool.tile([128, BPC, 2, N], bf16)

        def mult_reduce(b):
            prod = prod_pool.tile([128, 512], bf16)
            nc.vector.tensor_tensor(
                out=prod[:],
                in0=s1[:, b],
                in1=tab[:, b].rearrange("p c j -> p (c j)"),
                op=mybir.AluOpType.mult,
            )
            nc.vector.tensor_reduce(
                out=red_sb[:, b : b + 1],
                in_=prod[:],
                axis=mybir.AxisListType.X,
                op=mybir.AluOpType.add,
            )

        ptb_next = ptb_first
        for b in range(BPC):
            # casts are software-pipelined one batch ahead so the DVE queue
            # never stalls the next batch's matmuls
            ptb = ptb_next
            if b + 1 < BPC:
                ptb_next = ptb_pool.tile([128, 2, N], bf16)
                nc.vector.tensor_copy(ptb_next[:], pt_all[:, b + 1])
            if b > 0:
                mult_reduce(b - 1)
            if b == 3:
                nc.vector.tensor_copy(wlb[:], wl[:])

            # ---- QT = (P @ D)^T : QT[n, i] = sum_k D[k, n] * PT[k, i]
            qtb = qt_pool.tile([128, 2, N], bf16)
            for ncx in range(2):
                qt_ps = psum_pool.tile([128, N], f32, tag="qtps")
                for kc in range(2):
                    nc.tensor.matmul(
                        qt_ps[:],
                        lhsT=db[:, kc, ncx * 128 : (ncx + 1) * 128],
                        rhs=ptb[:, kc, :],
                        start=(kc == 0),
                        stop=(kc == 1),
                    )
                nc.scalar.copy(qtb[:, ncx, :], qt_ps[:])

            # ---- G: tab[p, b, c2, j] = G[128*c2 + p, j] (natural layout)
            for c2 in range(2):
                g_ps = psum_pool.tile([128, N], f32, tag="gps")
                for ncx in range(2):
                    nc.tensor.matmul(
                        g_ps[:],
                        lhsT=qtb[:, ncx, c2 * 128 : (c2 + 1) * 128],
                        rhs=ptb[:, ncx, :],
                        start=(ncx == 0),
                        stop=(ncx == 1),
                    )
                nc.scalar.copy(tab[:, b, c2], g_ps[:])

        mult_reduce(BPC - 1)

        # ---- leftover (rank>=2) edges: one gather over all batches
        gathl = const_pool.tile([128, nl], bf16)
        nc.gpsimd.indirect_copy(
            gathl[:],
            tab[:].rearrange("p b c j -> p (b c j)"),
            ixl[:].bitcast(u16),
            i_know_ap_gather_is_preferred=True,
        )
        prodl = const_pool.tile([128, BPC, nl], bf16)
        nc.vector.tensor_tensor(
            out=prodl[:],
            in0=gathl[:].unsqueeze(1).broadcast_to([128, BPC, nl]),
            in1=wlb[:],
            op=mybir.AluOpType.mult,
        )
        nc.vector.tensor_reduce(
            out=red_sb[:, BPC : 2 * BPC],
            in_=prodl[:],
            axis=mybir.AxisListType.X,
            op=mybir.AluOpType.add,
        )
        nc.vector.tensor_reduce(
            out=red_sb[:, 2 * BPC :],
            in_=ew_sb[:],
            axis=mybir.AxisListType.X,
            op=mybir.AluOpType.add,
        )

        # ---- cross-partition reduce of all partials in one matmul
        red_ps = psum_pool.tile([1, 3 * BPC], f32, tag="redps")
        nc.tensor.matmul(
            red_ps[:], lhsT=ones_sb[:], rhs=red_sb[:], start=True, stop=True
        )
        fin = const_pool.tile([1, 3 * BPC], f32)
        nc.vector.tensor_copy(fin[:], red_ps[:])

        # loss_b = (num_b + numleft_b) / max(sw_b, 1e-8); out = sum_b loss_b
        num = const_pool.tile([1, BPC], f32)
        nc.vector.tensor_tensor(
            out=num[:],
            in0=fin[:, :BPC],
            in1=fin[:, BPC : 2 * BPC],
            op=mybir.AluOpType.add,
        )
        sw_cl = const_pool.tile([1, BPC], f32)
        nc.vector.tensor_scalar_max(sw_cl[:], fin[:, 2 * BPC :], 1e-8)
        rsw = const_pool.tile([1, BPC], f32)
        nc.vector.reciprocal(rsw[:], sw_cl[:])
        lb = const_pool.tile([1, BPC], f32)
        nc.vector.tensor_tensor(
            out=lb[:], in0=num[:], in1=rsw[:], op=mybir.AluOpType.mult
        )
        tot = const_pool.tile([1, 1], f32)
        nc.vector.tensor_reduce(
            out=tot[:], in_=lb[:], axis=mybir.AxisListType.X, op=mybir.AluOpType.add
        )
        nc.sync.dma_start(out[:], tot[:])

    if not nc.is_finalized():
        nc.finalize()
    return nc


_NC_CACHE = {}


def _get_nc(ni1, ni2, nl):
    key = (ni1, ni2, nl)
    if key not in _NC_CACHE:
        _NC_CACHE[key] = _build_bass(ni1, ni2, nl)
    return _NC_CACHE[key]


def _cumcount(keys):
    """Occurrence rank of each element within equal values of sorted-able keys."""
    order = np.argsort(keys, kind="stable")
    sk = keys[order]
    is_new = np.r_[True, sk[1:] != sk[:-1]]
    starts = np.flatnonzero(is_new)
    n = len(keys)
    occ_sorted = np.arange(n) - np.repeat(starts, np.diff(np.r_[starts, n]))
    occ = np.empty(n, np.int64)
    occ[order] = occ_sorted
    return occ


def _round_up(x, m):
    return ((int(x) + m - 1) // m) * m


def _prep_in_maps(P, d_error, edge_i, edge_j, edge_w):
    P = np.asarray(P, dtype=np.float32)
    d_error = np.asarray(d_error, dtype=np.float32)
    edge_i = np.asarray(edge_i, dtype=np.int64)
    edge_j = np.asarray(edge_j, dtype=np.int64)
    edge_w = np.asarray(edge_w, dtype=np.float32)

    # P^T per batch, laid out [128, 2, N]: pt[b, p, c, :] = P[b, :, c*128+p]
    PT = np.ascontiguousarray(np.transpose(P, (0, 2, 1)))  # [B, N(k), N(i)]
    PT = np.ascontiguousarray(PT.reshape(B, 2, 128, N).transpose(0, 2, 1, 3))
    D = np.ascontiguousarray(d_error.reshape(2, 128, N).transpose(1, 0, 2))
    ew_l = np.ascontiguousarray(edge_w.reshape(B, Q, 128).transpose(0, 2, 1))

    # first pass over batches: compute ranks and required capacities
    meta = []
    need1 = need2 = 1
    lneed = np.zeros((NCORES, 8), np.int64)
    for b in range(B):
        i, jj, w = edge_i[b], edge_j[b], edge_w[b]
        p = i & 127
        cell = ((i >> 7) << 8) | jj
        rank = _cumcount((i << 8) | jj)
        meta.append((i, jj, w, p, cell, rank))
        if (rank == 0).any():
            need1 = max(need1, np.bincount(p[rank == 0]).max())
        if (rank == 1).any():
            need2 = max(need2, np.bincount(p[rank == 1]).max())
        m = rank >= 2
        if m.any():
            lneed[b // BPC] += np.bincount(p[m] >> 4, minlength=8)
    ni1 = max(NI1, _round_up(need1, 16))
    ni2 = max(NI2, _round_up(need2, 8))
    nl = max(NL, _round_up(lneed.max(), 64))

    ix1 = np.full((B, 128, ni1), -1, np.int16)
    dt1 = np.zeros((B, 128, ni1), np.float32)
    ix2 = np.full((B, 128, ni2), -1, np.int16)
    dt2 = np.zeros((B, 128, ni2), np.float32)
    # leftover, per core: slot stream per 16-partition group across batches
    ixl = np.zeros((NCORES, 128, nl // 16), np.int16)
    wl = np.zeros((NCORES, 128, BPC, nl), np.float32)
    lcount = np.zeros((NCORES, 8), np.int64)  # per core, per group

    for b in range(B):
        i, jj, w, p, cell, rank = meta[b]
        for pass_no, (ix, dt) in enumerate(((ix1, dt1), (ix2, dt2))):
            m = rank == pass_no
            pp, cc, ww = p[m], cell[m], w[m]
            k = _cumcount(pp)
            ix[b, pp, k] = (cc + 512 * (b % 2)).astype(np.int16)
            dt[b, pp, k] = ww

        m = rank >= 2
        pp, cc, ww = p[m], cell[m], w[m]
        core, bb = b // BPC, b % BPC
        g = pp >> 4
        for gg, ccc, www, ppp in zip(g, cc, ww, pp):
            s = lcount[core, gg]
            lcount[core, gg] += 1
            ixl[core, 16 * gg + (s % 16), s // 16] = bb * 512 + ccc
            wl[core, ppp, bb, s] = www

    in_maps = []
    for c in range(NCORES):
        sl = slice(c * BPC, (c + 1) * BPC)
        in_maps.append(
            {
                "pt": np.ascontiguousarray(PT[sl]),
                "derr": D,
                "ew": np.ascontiguousarray(ew_l[sl].transpose(1, 0, 2)),
                "ix1": np.ascontiguousarray(ix1[sl].transpose(1, 0, 2)),
                "dt1": np.ascontiguousarray(dt1[sl].transpose(1, 0, 2)),
                "ix2": np.ascontiguousarray(ix2[sl].transpose(1, 0, 2)),
                "dt2": np.ascontiguousarray(dt2[sl].transpose(1, 0, 2)),
                "ixl": np.ascontiguousarray(ixl[c]),
                "wl": np.ascontiguousarray(wl[c]),
            }
        )
    return in_maps, ni1, ni2, nl


def run(P, d_error, edge_i, edge_j, edge_w, trace=False):
    """Run on 8 cores; returns (loss_scalar, BassKernelResults)."""
    in_maps, ni1, ni2, nl = _prep_in_maps(P, d_error, edge_i, edge_j, edge_w)
    nc = _get_nc(ni1, ni2, nl)
    res = run_bass_kernel_spmd(
        nc, in_maps, core_ids=list(range(NCORES)), trace=trace
    )
    partials = [r["out"].reshape(()) for r in res.results]
    loss = np.float32(np.sum(np.stack(partials), dtype=np.float64) / B)
    return loss, res


def kernel(P, d_error, edge_i, edge_j, edge_w):
    loss, _ = run(P, d_error, edge_i, edge_j, edge_w, trace=False)
    return np.asarray(loss, dtype=np.float32)


# revision 34
# speedup vs baseline: 1.0684x; 1.0684x over previous
"""ErrorAwareEdgeLoss Trainium2 kernel (v5: local_scatter S-matrix).

Math: loss = mean_b [ (sum_e w_be * P[b,i_e,:] @ D @ P[b,j_e,:]) / max(sum_e w_be, 1e-8) ]

Reformulation:
    G_b = (P_b @ D) @ P_b^T               (two 256^3 matmuls on the PE, bf16)
    numerator_b = sum_e w_e * G_b[i_e, j_e] = <S_b, G_b>
    where S_b[i, j] = sum of w_e over edges with (i_e, j_e) = (i, j).

S_b is built ON-CHIP by gpsimd `local_scatter` (per-partition scatter through
Q7 local RAM, ~2us per 128x224-index call) in G's natural layout:
    S[p = i & 127, (i >> 7) * 256 + j] = w
local_scatter cannot accumulate duplicates, so edges are rank-split by
occurrence count of their exact (i, j): rank-0 edges go to table S1,
rank-1 to table S2 (separate scatter passes), and the rare rank>=2 edges
(~21 per batch) are handled on the gather side: one Pool-native
`indirect_copy` fetches G values for them from the bf16 G table and a
masked multiply adds their contribution. The scatters depend only on
host-prepared inputs, so they overlap the matmul pipeline entirely.

numerator = sum over cells of (S1+S2)*G  (DVE, 512 cells/partition/batch)
          + sum over leftover slots of w*gathered.

Sharding: data-parallel over batch: 8 NeuronCores x 8 batches. Each core
emits a partial sum of per-sample losses; the host adds the 8 partials and
divides by B (the all-reduce of the sharding hint).
"""

from contextlib import ExitStack

import numpy as np

import concourse.bacc as bacc
import concourse.bass as bass
import concourse.mybir as mybir
import concourse.tile as tile
from concourse.bass_utils import run_bass_kernel_spmd

B, N, E = 64, 256, 8192
NCORES = 8
BPC = B // NCORES  # batches per core
Q = E // 128  # ew free dim per partition (64)

NI1 = 112  # pass-1 scatter slots per partition per batch
NI2 = 24   # pass-2 slots per partition per batch
NL = 128   # leftover gather slots per 16-partition group (all batches)

f32 = mybir.dt.float32
bf16 = mybir.dt.bfloat16
i16 = mybir.dt.int16
u16 = mybir.dt.uint16


def _build_bass(ni1=None, ni2=None, nl=None):
    ni1, ni2, nl = ni1 or NI1, ni2 or NI2, nl or NL
    nc = bacc.Bacc("TRN2", target_bir_lowering=False, debug=False)

    pt_in = nc.dram_tensor("pt", [BPC, 128, 2, N], f32, kind="ExternalInput")
    d_in = nc.dram_tensor("derr", [128, 2, N], f32, kind="ExternalInput")
    ew_in = nc.dram_tensor("ew", [128, BPC, Q], f32, kind="ExternalInput")
    ix1_in = nc.dram_tensor("ix1", [128, BPC, ni1], i16, kind="ExternalInput")
    dt1_in = nc.dram_tensor("dt1", [128, BPC, ni1], f32, kind="ExternalInput")
    ix2_in = nc.dram_tensor("ix2", [128, BPC, ni2], i16, kind="ExternalInput")
    dt2_in = nc.dram_tensor("dt2", [128, BPC, ni2], f32, kind="ExternalInput")
    ixl_in = nc.dram_tensor("ixl", [128, nl // 16], i16, kind="ExternalInput")
    wl_in = nc.dram_tensor("wl", [128, BPC, nl], f32, kind="ExternalInput")
    out = nc.dram_tensor("out", [1, 1], f32, kind="ExternalOutput")

    with tile.TileContext(nc) as tc, ExitStack() as ctx:
        const_pool = ctx.enter_context(tc.tile_pool(name="const", bufs=1))
        pt_pool = ctx.enter_context(tc.tile_pool(name="pt", bufs=3))
        ptb_pool = ctx.enter_context(tc.tile_pool(name="ptb", bufs=3))
        qt_pool = ctx.enter_context(tc.tile_pool(name="qt", bufs=2))
        prod_pool = ctx.enter_context(tc.tile_pool(name="prod", bufs=2))
        psum_pool = ctx.enter_context(tc.tile_pool(name="ps", bufs=2, space="PSUM"))

        # ---- constants / whole-run tensors
        d_sb = const_pool.tile([128, 2, N], f32)
        nc.sync.dma_start(d_sb[:], d_in[:])
        db = const_pool.tile([128, 2, N], bf16)
        nc.vector.tensor_copy(db[:], d_sb[:])
        ones_sb = const_pool.tile([128, 1], f32)
        nc.vector.memset(ones_sb[:], 1.0)
        # cols [0,BPC): scatter numerators, [BPC,2B): leftover numerators,
        # [2B,3B): denominators
        red_sb = const_pool.tile([128, 3 * BPC], f32)

        ix1 = const_pool.tile([128, BPC, ni1], i16)
        dt1 = const_pool.tile([128, BPC, ni1], f32)
        ix2 = const_pool.tile([128, BPC, ni2], i16)
        dt2 = const_pool.tile([128, BPC, ni2], f32)
        ixl = const_pool.tile([128, nl // 16], i16)
        wl = const_pool.tile([128, BPC, nl], f32)
        ew_sb = const_pool.tile([128, BPC, Q], f32)
        pt_all = const_pool.tile([128, BPC, 2, N], f32)
        nc.sync.dma_start(pt_all[:, 0], pt_in[0])
        nc.sync.dma_start(ix1[:], ix1_in[:])
        nc.sync.dma_start(dt1[:], dt1_in[:])
        for b in range(1, BPC):
            nc.sync.dma_start(pt_all[:, b], pt_in[b])
        nc.sync.dma_start(ix2[:], ix2_in[:])
        nc.sync.dma_start(dt2[:], dt2_in[:])
        nc.sync.dma_start(ew_sb[:], ew_in[:])
        nc.sync.dma_start(ixl[:], ixl_in[:])
        nc.sync.dma_start(wl[:], wl_in[:])

        ptb_first = ptb_pool.tile([128, 2, N], bf16)
        nc.vector.tensor_copy(ptb_first[:], pt_all[:, 0])
        wlb = const_pool.tile([128, BPC, nl], bf16)
        db1 = const_pool.tile([128, BPC, ni1], bf16)
        nc.vector.tensor_copy(db1[:], dt1[:])
        db2 = const_pool.tile([128, BPC, ni2], bf16)
        nc.vector.tensor_copy(db2[:], dt2[:])

        # ---- scatter passes (input-only; overlap the matmul pipeline)
        s1 = const_pool.tile([128, BPC, 512], bf16)
        s2 = const_pool.tile([128, BPC, 512], bf16)
        for c in range(BPC // 2):
            nc.gpsimd.local_scatter(
                out_ap=s1[:, 2 * c : 2 * c + 2].rearrange("p b m -> p (b m)"),
                data_ap=db1[:, 2 * c : 2 * c + 2].rearrange("p b m -> p (b m)"),
                idxs_ap=ix1[:, 2 * c : 2 * c + 2].rearrange("p b m -> p (b m)"),
                channels=128,
                num_elems=1024,
                num_idxs=2 * ni1,
            )
            nc.gpsimd.local_scatter(
                out_ap=s2[:, 2 * c : 2 * c + 2].rearrange("p b m -> p (b m)"),
                data_ap=db2[:, 2 * c : 2 * c + 2].rearrange("p b m -> p (b m)"),
                idxs_ap=ix2[:, 2 * c : 2 * c + 2].rearrange("p b m -> p (b m)"),
                channels=128,
                num_elems=1024,
                num_idxs=2 * ni2,
            )
            # S += S2 for this 2-batch slice (in place)
            nc.vector.tensor_tensor(
                out=s1[:, 2 * c : 2 * c + 2],
                in0=s1[:, 2 * c : 2 * c + 2],
                in1=s2[:, 2 * c : 2 * c + 2],
                op=mybir.AluOpType.add,
            )

        nc.gpsimd.drain()

        tab = const_pool.tile([128, BPC, 2, N], bf16)

        def mult_reduce(b):
            prod = prod_pool.tile([128, 512], bf16)
            nc.vector.tensor_tensor(
                out=prod[:],
                in0=s1[:, b],
                in1=tab[:, b].rearrange("p c j -> p (c j)"),
                op=mybir.AluOpType.mult,
            )
            nc.vector.tensor_reduce(
                out=red_sb[:, b : b + 1],
                in_=prod[:],
                axis=mybir.AxisListType.X,
                op=mybir.AluOpType.add,
            )

        ptb_next = ptb_first
        for b in range(BPC):
            # casts are software-pipelined one batch ahead so the DVE queue
            # never stalls the next batch's matmuls
            ptb = ptb_next
            if b + 1 < BPC:
                ptb_next = ptb_pool.tile([128, 2, N], bf16)
                nc.vector.tensor_copy(ptb_next[:], pt_all[:, b + 1])
            if b > 0:
                mult_reduce(b - 1)
            if b == 3:
                nc.vector.tensor_copy(wlb[:], wl[:])
            if b == 6:
                nc.vector.tensor_reduce(
                    out=red_sb[:, 2 * BPC :],
                    in_=ew_sb[:],
                    axis=mybir.AxisListType.X,
                    op=mybir.AluOpType.add,
                )

            # ---- QT = (P @ D)^T : QT[n, i] = sum_k D[k, n] * PT[k, i]
            qtb = qt_pool.tile([128, 2, N], bf16)
            for ncx in range(2):
                qt_ps = psum_pool.tile([128, N], f32, tag="qtps")
                for kc in range(2):
                    nc.tensor.matmul(
                        qt_ps[:],
                        lhsT=db[:, kc, ncx * 128 : (ncx + 1) * 128],
                        rhs=ptb[:, kc, :],
                        start=(kc == 0),
                        stop=(kc == 1),
                    )
                nc.scalar.copy(qtb[:, ncx, :], qt_ps[:])

            # ---- G: tab[p, b, c2, j] = G[128*c2 + p, j] (natural layout)
            for c2 in range(2):
                g_ps = psum_pool.tile([128, N], f32, tag="gps")
                for ncx in range(2):
                    nc.tensor.matmul(
                        g_ps[:],
                        lhsT=qtb[:, ncx, c2 * 128 : (c2 + 1) * 128],
                        rhs=ptb[:, ncx, :],
                        start=(ncx == 0),
                        stop=(ncx == 1),
                    )
                nc.scalar.copy(tab[:, b, c2], g_ps[:])

        mult_reduce(BPC - 1)

        # ---- leftover (rank>=2) edges: one gather over all batches
        gathl = const_pool.tile([128, nl], bf16)
        nc.gpsimd.indirect_copy(
            gathl[:],
            tab[:].rearrange("p b c j -> p (b c j)"),
            ixl[:].bitcast(u16),
            i_know_ap_gather_is_preferred=True,
        )
        prodl = const_pool.tile([128, BPC, nl], bf16)
        nc.vector.tensor_tensor(
            out=prodl[:],
            in0=gathl[:].unsqueeze(1).broadcast_to([128, BPC, nl]),
            in1=wlb[:],
            op=mybir.AluOpType.mult,
        )
        nc.vector.tensor_reduce(
            out=red_sb[:, BPC : 2 * BPC],
            in_=prodl[:],
            axis=mybir.AxisListType.X,
            op=mybir.AluOpType.add,
        )
        # ---- cross-partition reduce of all partials in one matmul
        red_ps = psum_pool.tile([1, 3 * BPC], f32, tag="redps")
        nc.tensor.matmul(
            red_ps[:], lhsT=ones_sb[:], rhs=red_sb[:], start=True, stop=True
        )
        fin = const_pool.tile([1, 3 * BPC], f32)
        nc.vector.tensor_copy(fin[:], red_ps[:])

        # loss_b = (num_b + numleft_b) / max(sw_b, 1e-8); out = sum_b loss_b
        num = const_pool.tile([1, BPC], f32)
        nc.vector.tensor_tensor(
            out=num[:],
            in0=fin[:, :BPC],
            in1=fin[:, BPC : 2 * BPC],
            op=mybir.AluOpType.add,
        )
        sw_cl = const_pool.tile([1, BPC], f32)
        nc.vector.tensor_scalar_max(sw_cl[:], fin[:, 2 * BPC :], 1e-8)
        rsw = const_pool.tile([1, BPC], f32)
        nc.vector.reciprocal(rsw[:], sw_cl[:])
        lb = const_pool.tile([1, BPC], f32)
        nc.vector.tensor_tensor(
            out=lb[:], in0=num[:], in1=rsw[:], op=mybir.AluOpType.mult
        )
        tot = const_pool.tile([1, 1], f32)
        nc.vector.tensor_reduce(
            out=tot[:], in_=lb[:], axis=mybir.AxisListType.X, op=mybir.AluOpType.add
        )
        nc.sync.dma_start(out[:], tot[:])

    if not nc.is_finalized():
        nc.finalize()
    return nc


_NC_CACHE = {}


def _get_nc(ni1, ni2, nl):
    key = (ni1, ni2, nl)
    if key not in _NC_CACHE:
        _NC_CACHE[key] = _build_bass(ni1, ni2, nl)
    return _NC_CACHE[key]


def _cumcount(keys):
    """Occurrence rank of each element within equal values of sorted-able keys."""
    order = np.argsort(keys, kind="stable")
    sk = keys[order]
    is_new = np.r_[True, sk[1:] != sk[:-1]]
    starts = np.flatnonzero(is_new)
    n = len(keys)
    occ_sorted = np.arange(n) - np.repeat(starts, np.diff(np.r_[starts, n]))
    occ = np.empty(n, np.int64)
    occ[order] = occ_sorted
    return occ


def _round_up(x, m):
    return ((int(x) + m - 1) // m) * m


def _prep_in_maps(P, d_error, edge_i, edge_j, edge_w):
    P = np.asarray(P, dtype=np.float32)
    d_error = np.asarray(d_error, dtype=np.float32)
    edge_i = np.asarray(edge_i, dtype=np.int64)
    edge_j = np.asarray(edge_j, dtype=np.int64)
    edge_w = np.asarray(edge_w, dtype=np.float32)

    # P^T per batch, laid out [128, 2, N]: pt[b, p, c, :] = P[b, :, c*128+p]
    PT = np.ascontiguousarray(np.transpose(P, (0, 2, 1)))  # [B, N(k), N(i)]
    PT = np.ascontiguousarray(PT.reshape(B, 2, 128, N).transpose(0, 2, 1, 3))
    D = np.ascontiguousarray(d_error.reshape(2, 128, N).transpose(1, 0, 2))
    ew_l = np.ascontiguousarray(edge_w.reshape(B, Q, 128).transpose(0, 2, 1))

    # first pass over batches: compute ranks and required capacities
    meta = []
    need1 = need2 = 1
    lneed = np.zeros((NCORES, 8), np.int64)
    for b in range(B):
        i, jj, w = edge_i[b], edge_j[b], edge_w[b]
        p = i & 127
        cell = ((i >> 7) << 8) | jj
        rank = _cumcount((i << 8) | jj)
        meta.append((i, jj, w, p, cell, rank))
        if (rank == 0).any():
            need1 = max(need1, np.bincount(p[rank == 0]).max())
        if (rank == 1).any():
            need2 = max(need2, np.bincount(p[rank == 1]).max())
        m = rank >= 2
        if m.any():
            lneed[b // BPC] += np.bincount(p[m] >> 4, minlength=8)
    ni1 = max(NI1, _round_up(need1, 16))
    ni2 = max(NI2, _round_up(need2, 8))
    nl = max(NL, _round_up(lneed.max(), 64))

    ix1 = np.full((B, 128, ni1), -1, np.int16)
    dt1 = np.zeros((B, 128, ni1), np.float32)
    ix2 = np.full((B, 128, ni2), -1, np.int16)
    dt2 = np.zeros((B, 128, ni2), np.float32)
    # leftover, per core: slot stream per 16-partition group across batches
    ixl = np.zeros((NCORES, 128, nl // 16), np.int16)
    wl = np.zeros((NCORES, 128, BPC, nl), np.float32)
    lcount = np.zeros((NCORES, 8), np.int64)  # per core, per group

    for b in range(B):
        i, jj, w, p, cell, rank = meta[b]
        for pass_no, (ix, dt) in enumerate(((ix1, dt1), (ix2, dt2))):
            m = rank == pass_no
            pp, cc, ww = p[m], cell[m], w[m]
            k = _cumcount(pp)
            ix[b, pp, k] = (cc + 512 * (b % 2)).astype(np.int16)
            dt[b, pp, k] = ww

        m = rank >= 2
        pp, cc, ww = p[m], cell[m], w[m]
        core, bb = b // BPC, b % BPC
        g = pp >> 4
        for gg, ccc, www, ppp in zip(g, cc, ww, pp):
            s = lcount[core, gg]
            lcount[core, gg] += 1
            ixl[core, 16 * gg + (s % 16), s // 16] = bb * 512 + ccc
            wl[core, ppp, bb, s] = www

    in_maps = []
    for c in range(NCORES):
        sl = slice(c * BPC, (c + 1) * BPC)
        in_maps.append(
            {
                "pt": np.ascontiguousarray(PT[sl]),
                "derr": D,
                "ew": np.ascontiguousarray(ew_l[sl].transpose(1, 0, 2)),
                "ix1": np.ascontiguousarray(ix1[sl].transpose(1, 0, 2)),
                "dt1": np.ascontiguousarray(dt1[sl].transpose(1, 0, 2)),
                "ix2": np.ascontiguousarray(ix2[sl].transpose(1, 0, 2)),
                "dt2": np.ascontiguousarray(dt2[sl].transpose(1, 0, 2)),
                "ixl": np.ascontiguousarray(ixl[c]),
                "wl": np.ascontiguousarray(wl[c]),
            }
        )
    return in_maps, ni1, ni2, nl


def run(P, d_error, edge_i, edge_j, edge_w, trace=False):
    """Run on 8 cores; returns (loss_scalar, BassKernelResults)."""
    in_maps, ni1, ni2, nl = _prep_in_maps(P, d_error, edge_i, edge_j, edge_w)
    nc = _get_nc(ni1, ni2, nl)
    res = run_bass_kernel_spmd(
        nc, in_maps, core_ids=list(range(NCORES)), trace=trace
    )
    partials = [r["out"].reshape(()) for r in res.results]
    loss = np.float32(np.sum(np.stack(partials), dtype=np.float64) / B)
    return loss, res


def kernel(P, d_error, edge_i, edge_j, edge_w):
    loss, _ = run(P, d_error, edge_i, edge_j, edge_w, trace=False)
    return np.asarray(loss, dtype=np.float32)
